# revision 2
# baseline (speedup 1.0000x reference)
"""Chunked-attention Trainium2 kernel, v2 (8 NeuronCores, SPMD).

Reference computation (per batch b):
  q,k,v = x @ w{q,k,v}.T + b{q,k,v}            (H=16 heads, D=64)
  intra  = softmax(q k^T / sqrt(D)) v          within each 128-token chunk
  inter  = softmax(q k_means^T / sqrt(D)) v_means   chunk-causal over chunk means
  out    = (intra + inter) @ wo.T + bo

Sharding: 8 shards = (batch, seq-half).  Core c handles batch c//2, tokens
[half*4096, half*4096+4096).  Inter stage needs chunk means over the whole
batch-row; by linearity k_mean_j = xbar_j @ Wk, so the host ships the (64,1024)
chunk means of x and the device projects them - no cross-core communication.

Host-side algebraic folds (as baseline):
  - 1/sqrt(D) folded into Wq and bq;  bk dropped (softmax-invariant);
  - bv folded into bo (attention rows sum to 1):  bo_eff = bo + 2*(wo@bv);
  - no max-subtraction in softmax (scores ~N(0,1), fp32 exp safe).

v2 design (vs baseline):
  - q/k/v projections at N=512 moving (per 512-token superchunk): 4x fewer
    PE instructions; the PE sequencer was the baseline bottleneck.
  - scores computed s-major (key-token-major): S^T[s,t] = k_d,s q_d,t via
    k-stationary matmuls; chunk-causal mask applied as a per-partition bias
    on the exp (ACT), not as matmuls.
  - attention-out uses exp(S^T) slices as the STATIONARY operand:
    po_t[t,d] = sum_s At[s,t] v[s,d]  -> t-major output, so the softmax
    denominators are per-partition scalars: Z_intra[t,h] and Z_inter[t,h]
    come free from ones-columns embedded in vt/vm (65-wide head blocks), so
    no Z matmuls at all.  intra and inter are normalized SEPARATELY (two
    softmaxes in the reference): ao = poI*zinv_i + poJ*zinv_j on DVE
    (tensor_scalar + scalar_tensor_tensor).  The chunk-causal mask is a
    multiplicative per-partition mask on the inter exp block, applied on
    the idle GPSIMD engine.  No per-pair transposes, no DVE reductions.
  - one transpose pass per chunk: ao_t [t,E] -> aoT [E,t] (8 PE transposes
    + 8 DVE copies), interleaved into the next chunk's pair loop.
  - out-projection per superchunk at N=512 from aoT.

One NEFF for all 8 cores; per-core chunk causality comes in through the
masksT input (multiplicative 0/1 mask columns, [128, LCH] f32).
"""

import numpy as np
import ml_dtypes

import concourse.bass as bass
import concourse.mybir as mybir
import concourse.tile as tile
from concourse import bacc
from concourse.bass_utils import run_bass_kernel_spmd
from concourse.masks import make_identity
from concourse.alu_op_type import AluOpType

BF16 = mybir.dt.bfloat16
F32 = mybir.dt.float32
NPBF16 = ml_dtypes.bfloat16

B, S, E = 4, 8192, 1024
H, D, T = 16, 64, 128
C = S // T            # 64 chunks per batch row
N_CORES = 8
TOK = S // 2          # 4096 tokens per core
LCH = TOK // T        # 32 local chunks per core
SC_TOK = 512          # superchunk = 4 chunks
N_SC = TOK // SC_TOK  # 8
CH_PER_SC = SC_TOK // T
KT = E // 128         # k-tiles over the embed dim
MQ = E // 128         # m-tiles over q/k/out dims
VW = 65               # v block width per head: 64 data + 1 ones column
VTW = H * VW          # 1040 cols per chunk of v (16 heads)

Exp = mybir.ActivationFunctionType.Exp
Identity = mybir.ActivationFunctionType.Identity
Copy = mybir.ActivationFunctionType.Copy


def build_nc(n_sc: int = N_SC, repeat: int = 1):
    tok = n_sc * SC_TOK
    nc = bacc.Bacc("TRN2", debug=False, num_devices=N_CORES)
    xT = nc.dram_tensor("xT", (E, tok), BF16, kind="ExternalInput").ap()
    xbarT = nc.dram_tensor("xbarT", (E, C), BF16, kind="ExternalInput").ap()
    masksT = nc.dram_tensor("masksT", (128, LCH), F32, kind="ExternalInput").ap()
    wq = nc.dram_tensor("wq", (E, E), BF16, kind="ExternalInput").ap()
    wk = nc.dram_tensor("wk", (E, E), BF16, kind="ExternalInput").ap()
    wv = nc.dram_tensor("wv", (E, E), BF16, kind="ExternalInput").ap()
    wo = nc.dram_tensor("wo", (E, E), BF16, kind="ExternalInput").ap()
    bq = nc.dram_tensor("bq", (128, MQ), F32, kind="ExternalInput").ap()
    bo = nc.dram_tensor("bo", (128, MQ), F32, kind="ExternalInput").ap()
    outT = nc.dram_tensor("outT", (E, tok), F32, kind="ExternalOutput").ap()

    xT_r = xT.rearrange("(a p) t -> p a t", p=128)
    outT_r = outT.rearrange("(a p) t -> p a t", p=128)

    with tile.TileContext(nc) as tc:
        with (
            tc.tile_pool(name="singles", bufs=1) as singles,
            tc.tile_pool(name="scp", bufs=2) as scp,
            tc.tile_pool(name="chp", bufs=3) as chp,
            tc.tile_pool(name="atp", bufs=5) as atp,
            tc.tile_pool(name="zvp", bufs=6) as zvp,
            tc.tile_pool(name="ogp", bufs=3) as ogp,
            tc.tile_pool(name="psP", bufs=2, space="PSUM") as psP,
            tc.tile_pool(name="psS", bufs=3, space="PSUM") as psS,
            tc.tile_pool(name="psPo", bufs=2, space="PSUM") as psPo,
            tc.tile_pool(name="psT", bufs=1, space="PSUM") as psT,
        ):
            w_sb = {}
            # Each dma_start costs ~650ns of serial SP dispatch, so use few,
            # large DMAs, ordered by first use: wq+x for the first projection,
            # then mask (first chunk's ZAV), then wk/wv/xbar (k/v proj+means),
            # wo/bo last (not needed for tens of us).
            def _wdma(name, ap_):
                t = singles.tile([128, KT, E], BF16, tag=name, name=name)
                nc.sync.dma_start(out=t, in_=ap_.rearrange("(a p) f -> p a f", p=128))
                w_sb[name] = t

            wq_t = singles.tile([128, KT, E], BF16, tag="wq")
            wq_r = wq.rearrange("(a p) f -> p a f", p=128)
            xt0 = scp.tile([128, KT, SC_TOK], BF16, tag="xt")
            for a in range(KT):
                nc.sync.dma_start(out=wq_t[:, a, :], in_=wq_r[:, a, :])
                nc.sync.dma_start(out=xt0[:, a, :], in_=xT_r[:, a, 0:SC_TOK])
            w_sb["wq"] = wq_t
            bq_sb = singles.tile([128, MQ], F32, tag="bq")
            nc.sync.dma_start(out=bq_sb, in_=bq)
            mask_sb = singles.tile([128, LCH], F32, tag="mask")
            nc.sync.dma_start(out=mask_sb, in_=masksT)
            _wdma("wk", wk)
            _wdma("wv", wv)
            xbar_sb = singles.tile([128, KT, C], BF16, tag="xbar")
            nc.sync.dma_start(out=xbar_sb, in_=xbarT.rearrange("(a p) j -> p a j", p=128))
            _wdma("wo", wo)
            bo_sb = singles.tile([128, MQ], F32, tag="bo")
            nc.sync.dma_start(out=bo_sb, in_=bo)
            ident = singles.tile([128, 128], BF16, tag="ident")
            make_identity(nc, ident)

            pools = (singles, scp, chp, atp, zvp, ogp, psP, psS, psPo, psT)

            def body(_it=None):
                _body(nc, tc, pools, w_sb, bq_sb, bo_sb, xbar_sb, mask_sb,
                      ident, xT_r, outT_r, n_sc, xt0)

            if repeat == 1:
                body()
            else:
                with tc.For_i(0, repeat, 1) as _it:
                    body(_it)
    nc.compile()
    return nc


def _body(nc, tc, pools, w_sb, bq_sb, bo_sb, xbar_sb, mask_sb, ident,
          xT_r, outT_r, n_sc, xt0):
    (singles, scp, chp, atp, zvp, ogp, psP, psS, psPo, psT) = pools

    means = {}

    def emit_means():
        # chunk means of k and v, projected from the chunk means of x
        km_sb = singles.tile([128, MQ, C], BF16, tag="km")   # k_means^T (d-major)
        # v_means (j-major), duplicated in both partition halves so either
        # head of a pair can read it at its inter-At slice's base partition
        vm_sb = singles.tile([2 * C, VTW], BF16, tag="vm")
        vm_v = vm_sb.rearrange("p (h x) -> p h x", x=VW)
        nc.gpsimd.memset(vm_v[:, :, 64:65], 1.0)
        for m in range(MQ):
            pk_ = psP.tile([128, C], F32, tag="pj")
            for a in range(KT):
                nc.tensor.matmul(pk_, w_sb["wk"][:, a, m * 128:(m + 1) * 128],
                                 xbar_sb[:, a, :], start=(a == 0), stop=(a == KT - 1))
            nc.vector.tensor_copy(out=km_sb[:, m, :], in_=pk_)
        for n in range(2):
            pv_ = psP.tile([C, 512], F32, tag="pj")
            for a in range(KT):
                nc.tensor.matmul(pv_, xbar_sb[:, a, :],
                                 w_sb["wv"][:, a, n * 512:(n + 1) * 512],
                                 start=(a == 0), stop=(a == KT - 1))
            pv_v = pv_.rearrange("p (h d) -> p h d", d=64)
            nc.vector.tensor_copy(out=vm_v[0:C, 8 * n:8 * n + 8, 0:64], in_=pv_v)
            nc.vector.tensor_copy(out=vm_v[C:2 * C, 8 * n:8 * n + 8, 0:64], in_=pv_v)
        means["km"] = km_sb
        means["vm"] = vm_sb

    # --- per-SC tile state ---------------------------------------------
    cur = {}   # tiles for the SC currently being consumed by attention
    nxt = {}   # tiles for the next SC, being filled by interleaved proj

    def proj_groups(sc):
        """Yield closures, one proj group each, for superchunk sc."""
        xt = nxt["xt"]
        qT = nxt["qT"]
        kT = nxt["kT"]
        vt = nxt["vt"]

        for m in range(MQ):
            def g(m=m):
                pp = psP.tile([128, SC_TOK], F32, tag="pj")
                for a in range(KT):
                    nc.tensor.matmul(pp, w_sb["wq"][:, a, m * 128:(m + 1) * 128],
                                     xt[:, a, :], start=(a == 0), stop=(a == KT - 1))
                nc.vector.tensor_scalar_add(qT[:, m, :], pp, bq_sb[:, m:m + 1])
            yield g
        for m in range(MQ):
            def g(m=m):
                pp = psP.tile([128, SC_TOK], F32, tag="pj")
                for a in range(KT):
                    nc.tensor.matmul(pp, w_sb["wk"][:, a, m * 128:(m + 1) * 128],
                                     xt[:, a, :], start=(a == 0), stop=(a == KT - 1))
                nc.vector.tensor_copy(out=kT[:, m, :], in_=pp)
            yield g
        for cq in range(CH_PER_SC):
            for n in range(2):
                def g(cq=cq, n=n):
                    pp = psP.tile([128, 512], F32, tag="pj")
                    for a in range(KT):
                        nc.tensor.matmul(pp, xt[:, a, cq * T:(cq + 1) * T],
                                         w_sb["wv"][:, a, n * 512:(n + 1) * 512],
                                         start=(a == 0), stop=(a == KT - 1))
                    vt_v = vt.rearrange("p c (h x) -> p c h x", x=VW)
                    nc.vector.tensor_copy(
                        out=vt_v[:, cq, 8 * n:8 * n + 8, 0:64],
                        in_=pp.rearrange("p (h d) -> p h d", d=64))
                yield g

    def alloc_sc_tiles():
        t = {
            "xt": None,  # set by caller (xt0 or fresh DMA)
            "qT": scp.tile([128, MQ, SC_TOK], BF16, tag="qT", name="qT"),
            "kT": scp.tile([128, MQ, SC_TOK], BF16, tag="kT", name="kT"),
            "vt": scp.tile([128, CH_PER_SC, VTW], BF16, tag="vt", name="vt"),
            "aoT": scp.tile([128, KT, SC_TOK], BF16, tag="aoT", name="aoT"),
        }
        vt_v = t["vt"].rearrange("p c (h x) -> p c h x", x=VW)
        nc.gpsimd.memset(vt_v[:, :, :, 64:65], 1.0)
        return t

    def outproj_groups(aoT, sc, cols=slice(0, SC_TOK)):
        ncols = cols.stop - cols.start
        for mf in range(MQ):
            def g(mf=mf, aoT=aoT, sc=sc):
                pf = psP.tile([128, ncols], F32, tag="pj", name="pf")
                for a2 in range(KT):
                    nc.tensor.matmul(pf, w_sb["wo"][:, a2, mf * 128:(mf + 1) * 128],
                                     aoT[:, a2, cols], start=(a2 == 0), stop=(a2 == KT - 1))
                og = ogp.tile([128, ncols], F32, tag="og", name="og")
                nc.vector.tensor_scalar_add(og, pf, bo_sb[:, mf:mf + 1])
                nc.sync.dma_start(
                    out=outT_r[:, mf, sc * SC_TOK + cols.start:sc * SC_TOK + cols.stop],
                    in_=og)
            yield g

    # transpose tasks pending from the previous chunk: list of closures
    tr_pending = []
    # combined queue of big PE groups (next-SC projections + prev-SC
    # out-projections): exactly one is drained per pair slot
    grp_q = []
    # out-projections become available one chunk after their last transpose
    # batch was queued (the transposes drain during that chunk's pair loop)
    oproj_next = []
    aoT_by_sc = {}

    # ---- preamble: SC0 tiles + projections, means ----------------------
    nxt.update(alloc_sc_tiles())
    nxt["xt"] = xt0
    for g in proj_groups(0):
        g()
    emit_means()

    for c in range(n_sc * CH_PER_SC):
        sc, cq = divmod(c, CH_PER_SC)
        if cq == 0:
            # rotate SC state; prefetch x for the NEXT SC and set up its proj
            cur.clear()
            cur.update(nxt)
            nxt.clear()
            if sc + 1 < n_sc:
                nxt.update(alloc_sc_tiles())
                xt = scp.tile([128, KT, SC_TOK], BF16, tag="xt")
                nc.sync.dma_start(
                    out=xt, in_=xT_r[:, :, (sc + 1) * SC_TOK:(sc + 2) * SC_TOK])
                nxt["xt"] = xt
                grp_q.extend(proj_groups(sc + 1))
        if cq == 1 and oproj_next:
            grp_q.extend(outproj_groups(*oproj_next.pop(0)))

        km_sb = means["km"]
        vm_sb = means["vm"]
        qT, kT, vt, aoT = cur["qT"], cur["kT"], cur["vt"], cur["aoT"]
        ts_ = slice(cq * T, (cq + 1) * T)

        ao_t = chp.tile([128, E], BF16, tag="ao_t")
        # po tile: one pair = 2 heads x [64 intra | Zi | 64 inter | Zj] f32;
        # Z comes from the ones columns embedded in vt/vm (no Z matmuls)
        pair_state = {}  # p -> (po, zinv)
        zav_q = []       # deferred-by-two-pairs ZAV list

        def emit_zav(p, At):
            po = psPo.tile([128, 2 * VW * 2], F32, tag="po", name="po")
            for sub in range(2):
                h = 2 * p + sub
                hb = sub * 2 * VW
                a_i = At[:, sub * T:(sub + 1) * T]                # intra, [s=128, t]
                a_j = At[64 * sub:64 * sub + 64, 2 * T:3 * T]     # inter, [j=64, t]
                vslice = vt[:, cq, h * VW:(h + 1) * VW]
                vmslice = vm_sb[C * sub:C * (sub + 1), h * VW:(h + 1) * VW]
                nc.tensor.matmul(po[:, hb:hb + VW], a_i, vslice,
                                 start=True, stop=True)
                nc.tensor.matmul(po[:, hb + VW:hb + 2 * VW], a_j, vmslice,
                                 start=True, stop=True)
            zinv = zvp.tile([128, 4], F32, tag="zinv", name="zinv")
            nc.vector.reciprocal(zinv, po.rearrange("p (h x) -> p h x", x=VW)[:, :, 64])
            pair_state[p] = (po, zinv)
            if p >= 1:
                emit_norms(p - 1)

        def emit_norms(p):
            po, zinv = pair_state.pop(p)
            for sub in range(2):
                h = 2 * p + sub
                hb = sub * 2 * VW
                sl = slice(h * 64, (h + 1) * 64)
                nc.vector.tensor_scalar_mul(
                    ao_t[:, sl], po[:, hb:hb + 64],
                    zinv[:, 2 * sub:2 * sub + 1])
                nc.vector.scalar_tensor_tensor(
                    out=ao_t[:, sl], in0=po[:, hb + VW:hb + VW + 64],
                    scalar=zinv[:, 2 * sub + 1:2 * sub + 2], in1=ao_t[:, sl],
                    op0=AluOpType.mult, op1=AluOpType.add)

        for p in range(MQ):
            # scores, s-major: [s|s|j-stack] x t
            sc_ps = psS.tile([128, 3 * T], F32, tag="sc")
            for sub in range(2):
                rs = slice(64 * sub, 64 * sub + 64)
                nc.tensor.matmul(sc_ps[:, sub * T:(sub + 1) * T],
                                 kT[rs, p, ts_], qT[rs, p, ts_],
                                 start=True, stop=True)
                nc.tensor.matmul(sc_ps[rs, 2 * T:3 * T],
                                 km_sb[rs, p, :], qT[rs, p, ts_],
                                 start=True, stop=True)
            At = atp.tile([128, 3 * T], BF16, tag="At")
            nc.scalar.activation(At, sc_ps, Exp)
            # chunk-causal mask: zero the masked j rows of the inter block
            # (per-partition multiplicative mask, on the idle GPSIMD engine)
            nc.gpsimd.tensor_scalar_mul(At[:, 2 * T:3 * T], At[:, 2 * T:3 * T],
                                        mask_sb[:, c:c + 1])

            # keep PE busy between scores(p) and ZAV(p): one big matmul group
            # (next-SC projection or prev-SC out-projection), plus one
            # pending transpose of the previous chunk
            if tr_pending:
                tr_pending.pop(0)()
            # 24 groups arrive per 32 pair slots: pace 3-of-4 so the last
            # chunk of each SC still gets PE filler between scores and ZAV
            if grp_q and (p % 4 != 3 or len(grp_q) > 8):
                grp_q.pop(0)()

            zav_q.append((p, At))
            if len(zav_q) > 2:
                emit_zav(*zav_q.pop(0))
        while zav_q:
            emit_zav(*zav_q.pop(0))
        emit_norms(MQ - 1)
        while tr_pending:
            tr_pending.pop(0)()
        # last SC: out-project per chunk (the chunk whose transposes just
        # drained) so the tail isn't one big serial out-projection
        pc = c - 1
        if pc >= 0 and pc // CH_PER_SC == n_sc - 1:
            psc, pcq = divmod(pc, CH_PER_SC)
            grp_q.extend(outproj_groups(aoT_by_sc[psc], psc,
                                        slice(pcq * T, (pcq + 1) * T)))

        def make_tr(a, ao_t=ao_t, aoT=aoT, cq=cq):
            def tr():
                ptr = psT.tile([128, 128], BF16, tag="tr")
                nc.tensor.transpose(ptr, ao_t[:, a * 128:(a + 1) * 128], ident)
                nc.scalar.activation(aoT[:, a, cq * T:(cq + 1) * T], ptr, Copy)
            return tr
        tr_pending.extend(make_tr(a) for a in range(KT))

        if cq == 0 and sc == n_sc - 1:
            aoT_by_sc[sc] = aoT
        if cq == CH_PER_SC - 1 and sc < n_sc - 1:
            oproj_next.append((aoT, sc))

    # drain: last chunk's transposes interleaved with the already-available
    # out-projection groups, then the final two chunks' out-projections
    while tr_pending:
        tr_pending.pop(0)()
        if grp_q:
            grp_q.pop(0)()
    last_c = n_sc * CH_PER_SC - 1
    fsc, fcq = divmod(last_c, CH_PER_SC)
    grp_q.extend(outproj_groups(aoT_by_sc[fsc], fsc,
                                slice(fcq * T, (fcq + 1) * T)))
    while oproj_next:
        grp_q.extend(outproj_groups(*oproj_next.pop(0)))
    while grp_q:
        grp_q.pop(0)()


def host_prep(hidden_states, wq, bq, wk, bk, wv, bv, wo, bo):
    """Per-core input maps (list of 8 dicts) from the full fp32 inputs."""
    x = np.asarray(hidden_states, dtype=np.float32)
    scale = 1.0 / np.sqrt(D)
    Wq = (np.asarray(wq).T * scale).astype(NPBF16)
    Wk = np.asarray(wk).T.astype(NPBF16)
    Wv = np.asarray(wv).T.astype(NPBF16)
    Wo = np.asarray(wo).T.astype(NPBF16)
    bq_eff = np.ascontiguousarray((np.asarray(bq) * scale).reshape(MQ, 128).T).astype(np.float32)
    bo_eff = np.ascontiguousarray(
        (np.asarray(bo) + 2.0 * (np.asarray(wo) @ np.asarray(bv))).reshape(MQ, 128).T
    ).astype(np.float32)
    xbar = x.reshape(B, C, T, E).mean(axis=2)  # (B, C, E) fp32

    j_half = np.arange(C) % C  # j index vector
    cl_idx = np.arange(LCH)[None, :]
    in_maps = []
    for c in range(N_CORES):
        b, half = divmod(c, 2)
        xs = x[b, half * TOK:(half + 1) * TOK, :]
        # masksT[j_stacked, c_loc]: additive bias on inter scores, per
        # partition j (rows 0:64 = head-even j, rows 64:128 = head-odd j)
        jj = np.arange(64)[:, None]
        m64 = np.where(jj <= half * LCH + cl_idx, 1.0, 0.0)
        mT = np.concatenate([m64, m64], axis=0).astype(np.float32)  # (128, LCH)
        in_maps.append({
            "xT": xs.T.astype(NPBF16),
            "xbarT": xbar[b].T.astype(NPBF16),
            "masksT": mT,
            "wq": Wq, "wk": Wk, "wv": Wv, "wo": Wo,
            "bq": bq_eff, "bo": bo_eff,
        })
    return in_maps


_NC_CACHE = {}


def _get_nc():
    if "nc" not in _NC_CACHE:
        _NC_CACHE["nc"] = build_nc(N_SC)
    return _NC_CACHE["nc"]


def kernel(**inputs):
    in_maps = host_prep(**inputs)
    nc = _get_nc()
    res = run_bass_kernel_spmd(nc, in_maps, core_ids=list(range(N_CORES)))
    out = np.empty((B, S, E), dtype=np.float32)
    for c in range(N_CORES):
        b, half = divmod(c, 2)
        out[b, half * TOK:(half + 1) * TOK, :] = res.results[c]["outT"].T
    return out


# revision 3
# speedup vs baseline: 1.1387x; 1.1387x over previous
"""Chunked-attention Trainium2 kernel, v2 (8 NeuronCores, SPMD).

Reference computation (per batch b):
  q,k,v = x @ w{q,k,v}.T + b{q,k,v}            (H=16 heads, D=64)
  intra  = softmax(q k^T / sqrt(D)) v          within each 128-token chunk
  inter  = softmax(q k_means^T / sqrt(D)) v_means   chunk-causal over chunk means
  out    = (intra + inter) @ wo.T + bo

Sharding: 8 shards = (batch, seq-half).  Core c handles batch c//2, tokens
[half*4096, half*4096+4096).  Inter stage needs chunk means over the whole
batch-row; by linearity k_mean_j = xbar_j @ Wk, so the host ships the (64,1024)
chunk means of x and the device projects them - no cross-core communication.

Host-side algebraic folds (as baseline):
  - 1/sqrt(D) folded into Wq and bq;  bk dropped (softmax-invariant);
  - bv folded into bo (attention rows sum to 1):  bo_eff = bo + 2*(wo@bv);
  - no max-subtraction in softmax (scores ~N(0,1), fp32 exp safe).

v2 design (vs baseline):
  - q/k/v projections at N=512 moving (per 512-token superchunk): 4x fewer
    PE instructions; the PE sequencer was the baseline bottleneck.
  - scores computed s-major (key-token-major): S^T[s,t] = k_d,s q_d,t via
    k-stationary matmuls; chunk-causal mask applied as a per-partition bias
    on the exp (ACT), not as matmuls.
  - attention-out uses exp(S^T) slices as the STATIONARY operand:
    po_t[t,d] = sum_s At[s,t] v[s,d]  -> t-major output, so the softmax
    denominators are per-partition scalars: Z_intra[t,h] and Z_inter[t,h]
    come free from ones-columns embedded in vt/vm (65-wide head blocks), so
    no Z matmuls at all.  intra and inter are normalized SEPARATELY (two
    softmaxes in the reference): ao = poI*zinv_i + poJ*zinv_j on DVE
    (tensor_scalar + scalar_tensor_tensor).  The chunk-causal mask is a
    multiplicative per-partition mask on the inter exp block, applied on
    the idle GPSIMD engine.  No per-pair transposes, no DVE reductions.
  - one transpose pass per chunk: ao_t [t,E] -> aoT [E,t] (8 PE transposes
    + 8 DVE copies), interleaved into the next chunk's pair loop.
  - out-projection per superchunk at N=512 from aoT.

One NEFF for all 8 cores; per-core chunk causality comes in through the
masksT input (multiplicative 0/1 mask columns, [128, LCH] f32).
"""

import numpy as np
import ml_dtypes

import concourse.bass as bass
import concourse.mybir as mybir
import concourse.tile as tile
from concourse import bacc
from concourse.bass_utils import run_bass_kernel_spmd
from concourse.masks import make_identity
from concourse.alu_op_type import AluOpType

BF16 = mybir.dt.bfloat16
F32 = mybir.dt.float32
NPBF16 = ml_dtypes.bfloat16

B, S, E = 4, 8192, 1024
H, D, T = 16, 64, 128
C = S // T            # 64 chunks per batch row
N_CORES = 8
TOK = S // 2          # 4096 tokens per core
LCH = TOK // T        # 32 local chunks per core
SC_TOK = 512          # superchunk = 4 chunks
N_SC = TOK // SC_TOK  # 8
CH_PER_SC = SC_TOK // T
KT = E // 128         # k-tiles over the embed dim
MQ = E // 128         # m-tiles over q/k/out dims
VW = 65               # v block width per head: 64 data + 1 ones column
VTW = H * VW          # 1040 cols per chunk of v (16 heads)

Exp = mybir.ActivationFunctionType.Exp
Identity = mybir.ActivationFunctionType.Identity
Copy = mybir.ActivationFunctionType.Copy


def build_nc(n_sc: int = N_SC, repeat: int = 1):
    tok = n_sc * SC_TOK
    nc = bacc.Bacc("TRN2", debug=False, num_devices=N_CORES)
    xT = nc.dram_tensor("xT", (E, tok), BF16, kind="ExternalInput").ap()
    xbarT = nc.dram_tensor("xbarT", (E, C), BF16, kind="ExternalInput").ap()
    masksT = nc.dram_tensor("masksT", (128, LCH), F32, kind="ExternalInput").ap()
    wq = nc.dram_tensor("wq", (E, E), BF16, kind="ExternalInput").ap()
    wk = nc.dram_tensor("wk", (E, E), BF16, kind="ExternalInput").ap()
    wv = nc.dram_tensor("wv", (E, E), BF16, kind="ExternalInput").ap()
    wo = nc.dram_tensor("wo", (E, E), BF16, kind="ExternalInput").ap()
    bq = nc.dram_tensor("bq", (128, MQ), F32, kind="ExternalInput").ap()
    bo = nc.dram_tensor("bo", (128, MQ), F32, kind="ExternalInput").ap()
    outT = nc.dram_tensor("outT", (E, tok), F32, kind="ExternalOutput").ap()

    xT_r = xT.rearrange("(a p) t -> p a t", p=128)
    outT_r = outT.rearrange("(a p) t -> p a t", p=128)

    with tile.TileContext(nc) as tc:
        with (
            tc.tile_pool(name="singles", bufs=1) as singles,
            tc.tile_pool(name="scp", bufs=2) as scp,
            tc.tile_pool(name="chp", bufs=3) as chp,
            tc.tile_pool(name="atp", bufs=5) as atp,
            tc.tile_pool(name="zvp", bufs=6) as zvp,
            tc.tile_pool(name="ogp", bufs=3) as ogp,
            tc.tile_pool(name="psP", bufs=2, space="PSUM") as psP,
            tc.tile_pool(name="psS", bufs=2, space="PSUM") as psS,
            tc.tile_pool(name="psPo", bufs=3, space="PSUM") as psPo,
            tc.tile_pool(name="psT", bufs=1, space="PSUM") as psT,
        ):
            w_sb = {}
            # Each dma_start costs ~650ns of serial SP dispatch, so use few,
            # large DMAs, ordered by first use: wq+x for the first projection,
            # then mask (first chunk's ZAV), then wk/wv/xbar (k/v proj+means),
            # wo/bo last (not needed for tens of us).
            def _wdma(name, ap_):
                t = singles.tile([128, KT, E], BF16, tag=name, name=name)
                nc.sync.dma_start(out=t, in_=ap_.rearrange("(a p) f -> p a f", p=128))
                w_sb[name] = t

            wq_t = singles.tile([128, KT, E], BF16, tag="wq")
            wq_r = wq.rearrange("(a p) f -> p a f", p=128)
            xt0 = scp.tile([128, KT, SC_TOK], BF16, tag="xt")
            for a in range(KT):
                nc.sync.dma_start(out=wq_t[:, a, :], in_=wq_r[:, a, :])
                nc.sync.dma_start(out=xt0[:, a, :], in_=xT_r[:, a, 0:SC_TOK])
            w_sb["wq"] = wq_t
            bq_sb = singles.tile([128, MQ], F32, tag="bq")
            nc.sync.dma_start(out=bq_sb, in_=bq)
            mask_sb = singles.tile([128, LCH], F32, tag="mask")
            nc.sync.dma_start(out=mask_sb, in_=masksT)
            _wdma("wk", wk)
            _wdma("wv", wv)
            xbar_sb = singles.tile([128, KT, C], BF16, tag="xbar")
            nc.sync.dma_start(out=xbar_sb, in_=xbarT.rearrange("(a p) j -> p a j", p=128))
            _wdma("wo", wo)
            bo_sb = singles.tile([128, MQ], F32, tag="bo")
            nc.sync.dma_start(out=bo_sb, in_=bo)
            ident = singles.tile([128, 128], BF16, tag="ident")
            make_identity(nc, ident)

            pools = (singles, scp, chp, atp, zvp, ogp, psP, psS, psPo, psT)

            def body(_it=None):
                _body(nc, tc, pools, w_sb, bq_sb, bo_sb, xbar_sb, mask_sb,
                      ident, xT_r, outT_r, n_sc, xt0)

            if repeat == 1:
                body()
            else:
                with tc.For_i(0, repeat, 1) as _it:
                    body(_it)
    nc.compile()
    return nc


def _body(nc, tc, pools, w_sb, bq_sb, bo_sb, xbar_sb, mask_sb, ident,
          xT_r, outT_r, n_sc, xt0):
    (singles, scp, chp, atp, zvp, ogp, psP, psS, psPo, psT) = pools

    means = {}

    def emit_means():
        # chunk means of k and v, projected from the chunk means of x
        km_sb = singles.tile([128, MQ, C], BF16, tag="km")   # k_means^T (d-major)
        # v_means (j-major), duplicated in both partition halves so either
        # head of a pair can read it at its inter-At slice's base partition
        vm_sb = singles.tile([2 * C, VTW], BF16, tag="vm")
        vm_v = vm_sb.rearrange("p (h x) -> p h x", x=VW)
        nc.gpsimd.memset(vm_v[:, :, 64:65], 1.0)
        for m in range(MQ):
            pk_ = psP.tile([128, C], F32, tag="pj")
            for a in range(KT):
                nc.tensor.matmul(pk_, w_sb["wk"][:, a, m * 128:(m + 1) * 128],
                                 xbar_sb[:, a, :], start=(a == 0), stop=(a == KT - 1))
            nc.vector.tensor_copy(out=km_sb[:, m, :], in_=pk_)
        for n in range(2):
            pv_ = psP.tile([C, 512], F32, tag="pj")
            for a in range(KT):
                nc.tensor.matmul(pv_, xbar_sb[:, a, :],
                                 w_sb["wv"][:, a, n * 512:(n + 1) * 512],
                                 start=(a == 0), stop=(a == KT - 1))
            pv_v = pv_.rearrange("p (h d) -> p h d", d=64)
            nc.vector.tensor_copy(out=vm_v[0:C, 8 * n:8 * n + 8, 0:64], in_=pv_v)
            nc.vector.tensor_copy(out=vm_v[C:2 * C, 8 * n:8 * n + 8, 0:64], in_=pv_v)
        means["km"] = km_sb
        means["vm"] = vm_sb

    # --- per-SC tile state ---------------------------------------------
    cur = {}   # tiles for the SC currently being consumed by attention
    nxt = {}   # tiles for the next SC, being filled by interleaved proj

    def proj_groups(sc):
        """Yield closures, one proj group each, for superchunk sc."""
        xt = nxt["xt"]
        qT = nxt["qT"]
        kT = nxt["kT"]
        vt = nxt["vt"]

        for m in range(MQ):
            def g(m=m):
                pp = psP.tile([128, SC_TOK], F32, tag="pj")
                for a in range(KT):
                    nc.tensor.matmul(pp, w_sb["wq"][:, a, m * 128:(m + 1) * 128],
                                     xt[:, a, :], start=(a == 0), stop=(a == KT - 1))
                nc.vector.tensor_scalar_add(qT[:, m, :], pp, bq_sb[:, m:m + 1])
            yield g
        for m in range(MQ):
            def g(m=m):
                pp = psP.tile([128, SC_TOK], F32, tag="pj")
                for a in range(KT):
                    nc.tensor.matmul(pp, w_sb["wk"][:, a, m * 128:(m + 1) * 128],
                                     xt[:, a, :], start=(a == 0), stop=(a == KT - 1))
                nc.vector.tensor_copy(out=kT[:, m, :], in_=pp)
            yield g
        for cq in range(CH_PER_SC):
            for n in range(2):
                def g(cq=cq, n=n):
                    pp = psP.tile([128, 512], F32, tag="pj")
                    for a in range(KT):
                        nc.tensor.matmul(pp, xt[:, a, cq * T:(cq + 1) * T],
                                         w_sb["wv"][:, a, n * 512:(n + 1) * 512],
                                         start=(a == 0), stop=(a == KT - 1))
                    vt_v = vt.rearrange("p c (h x) -> p c h x", x=VW)
                    nc.vector.tensor_copy(
                        out=vt_v[:, cq, 8 * n:8 * n + 8, 0:64],
                        in_=pp.rearrange("p (h d) -> p h d", d=64))
                yield g

    def alloc_sc_tiles():
        t = {
            "xt": None,  # set by caller (xt0 or fresh DMA)
            "qT": scp.tile([128, MQ, SC_TOK], BF16, tag="qT", name="qT"),
            "kT": scp.tile([128, MQ, SC_TOK], BF16, tag="kT", name="kT"),
            "vt": scp.tile([128, CH_PER_SC, VTW], BF16, tag="vt", name="vt"),
            "aoT": scp.tile([128, KT, SC_TOK], BF16, tag="aoT", name="aoT"),
        }
        vt_v = t["vt"].rearrange("p c (h x) -> p c h x", x=VW)
        nc.gpsimd.memset(vt_v[:, :, :, 64:65], 1.0)
        return t

    def outproj_groups(aoT, sc, cols=slice(0, SC_TOK)):
        ncols = cols.stop - cols.start
        for mf in range(MQ):
            def g(mf=mf, aoT=aoT, sc=sc):
                pf = psP.tile([128, ncols], F32, tag="pj", name="pf")
                for a2 in range(KT):
                    nc.tensor.matmul(pf, w_sb["wo"][:, a2, mf * 128:(mf + 1) * 128],
                                     aoT[:, a2, cols], start=(a2 == 0), stop=(a2 == KT - 1))
                og = ogp.tile([128, ncols], F32, tag="og", name="og")
                nc.vector.tensor_scalar_add(og, pf, bo_sb[:, mf:mf + 1])
                nc.sync.dma_start(
                    out=outT_r[:, mf, sc * SC_TOK + cols.start:sc * SC_TOK + cols.stop],
                    in_=og)
            yield g

    # transpose tasks pending from the previous chunk: list of closures
    tr_pending = []
    # combined queue of big PE groups (next-SC projections + prev-SC
    # out-projections): exactly one is drained per pair slot
    grp_q = []
    # out-projections become available one chunk after their last transpose
    # batch was queued (the transposes drain during that chunk's pair loop)
    oproj_next = []
    aoT_by_sc = {}

    # ---- preamble: SC0 tiles + projections, means ----------------------
    nxt.update(alloc_sc_tiles())
    nxt["xt"] = xt0
    for g in proj_groups(0):
        g()
    emit_means()

    for c in range(n_sc * CH_PER_SC):
        sc, cq = divmod(c, CH_PER_SC)
        if cq == 0:
            # rotate SC state; prefetch x for the NEXT SC and set up its proj
            cur.clear()
            cur.update(nxt)
            nxt.clear()
            if sc + 1 < n_sc:
                nxt.update(alloc_sc_tiles())
                xt = scp.tile([128, KT, SC_TOK], BF16, tag="xt")
                nc.sync.dma_start(
                    out=xt, in_=xT_r[:, :, (sc + 1) * SC_TOK:(sc + 2) * SC_TOK])
                nxt["xt"] = xt
                grp_q.extend(proj_groups(sc + 1))
        if cq == 1 and oproj_next:
            grp_q.extend(outproj_groups(*oproj_next.pop(0)))

        km_sb = means["km"]
        vm_sb = means["vm"]
        qT, kT, vt, aoT = cur["qT"], cur["kT"], cur["vt"], cur["aoT"]
        ts_ = slice(cq * T, (cq + 1) * T)

        ao_t = chp.tile([128, E], BF16, tag="ao_t")
        # po tile: one pair = 2 heads x [64 intra | Zi | 64 inter | Zj] f32;
        # Z comes from the ones columns embedded in vt/vm (no Z matmuls)
        pair_state = {}  # p -> (po, zinv)
        zav_q = []       # deferred-by-two-pairs ZAV list

        def emit_zav(p, At):
            po = psPo.tile([128, 2 * VW * 2], F32, tag="po", name="po")
            for sub in range(2):
                h = 2 * p + sub
                hb = sub * 2 * VW
                a_i = At[:, sub * T:(sub + 1) * T]                # intra, [s=128, t]
                a_j = At[64 * sub:64 * sub + 64, 2 * T:3 * T]     # inter, [j=64, t]
                vslice = vt[:, cq, h * VW:(h + 1) * VW]
                vmslice = vm_sb[C * sub:C * (sub + 1), h * VW:(h + 1) * VW]
                nc.tensor.matmul(po[:, hb:hb + VW], a_i, vslice,
                                 start=True, stop=True)
                nc.tensor.matmul(po[:, hb + VW:hb + 2 * VW], a_j, vmslice,
                                 start=True, stop=True)
            zinv = zvp.tile([128, 4], F32, tag="zinv", name="zinv")
            nc.vector.reciprocal(zinv, po.rearrange("p (h x) -> p h x", x=VW)[:, :, 64])
            pair_state[p] = (po, zinv)
            if p >= 1:
                emit_norms(p - 1)

        def emit_norms(p):
            po, zinv = pair_state.pop(p)
            for sub in range(2):
                h = 2 * p + sub
                hb = sub * 2 * VW
                sl = slice(h * 64, (h + 1) * 64)
                nc.scalar.activation(ao_t[:, sl], po[:, hb:hb + 64], Identity,
                                     scale=zinv[:, 2 * sub:2 * sub + 1])
                nc.vector.scalar_tensor_tensor(
                    out=ao_t[:, sl], in0=po[:, hb + VW:hb + VW + 64],
                    scalar=zinv[:, 2 * sub + 1:2 * sub + 2], in1=ao_t[:, sl],
                    op0=AluOpType.mult, op1=AluOpType.add)

        for p in range(MQ):
            # scores, s-major: [s|s|j-stack] x t
            sc_ps = psS.tile([128, 3 * T], F32, tag="sc")
            for sub in range(2):
                rs = slice(64 * sub, 64 * sub + 64)
                nc.tensor.matmul(sc_ps[:, sub * T:(sub + 1) * T],
                                 kT[rs, p, ts_], qT[rs, p, ts_],
                                 start=True, stop=True)
                nc.tensor.matmul(sc_ps[rs, 2 * T:3 * T],
                                 km_sb[rs, p, :], qT[rs, p, ts_],
                                 start=True, stop=True)
            At = atp.tile([128, 3 * T], BF16, tag="At")
            nc.scalar.activation(At, sc_ps, Exp)
            # chunk-causal mask: zero the masked j rows of the inter block
            # (per-partition multiplicative mask, on the idle GPSIMD engine)
            nc.gpsimd.tensor_scalar_mul(At[:, 2 * T:3 * T], At[:, 2 * T:3 * T],
                                        mask_sb[:, c:c + 1])

            # keep PE busy between scores(p) and ZAV(p): one big matmul group
            # (next-SC projection or prev-SC out-projection), plus one
            # pending transpose of the previous chunk
            if tr_pending:
                tr_pending.pop(0)()
            # 24 groups arrive per 32 pair slots: pace 3-of-4 so the last
            # chunk of each SC still gets PE filler between scores and ZAV
            if grp_q and (p % 4 != 3 or len(grp_q) > 8):
                grp_q.pop(0)()

            zav_q.append((p, At))
            if len(zav_q) > 2:
                emit_zav(*zav_q.pop(0))
        while zav_q:
            emit_zav(*zav_q.pop(0))
        emit_norms(MQ - 1)
        while tr_pending:
            tr_pending.pop(0)()
        # last SC: out-project per chunk (the chunk whose transposes just
        # drained) so the tail isn't one big serial out-projection
        pc = c - 1
        if pc >= 0 and pc // CH_PER_SC == n_sc - 1:
            psc, pcq = divmod(pc, CH_PER_SC)
            grp_q.extend(outproj_groups(aoT_by_sc[psc], psc,
                                        slice(pcq * T, (pcq + 1) * T)))

        def make_tr(a, ao_t=ao_t, aoT=aoT, cq=cq):
            def tr():
                ptr = psT.tile([128, 128], BF16, tag="tr")
                nc.tensor.transpose(ptr, ao_t[:, a * 128:(a + 1) * 128], ident)
                nc.scalar.activation(aoT[:, a, cq * T:(cq + 1) * T], ptr, Copy)
            return tr
        tr_pending.extend(make_tr(a) for a in range(KT))

        if cq == 0 and sc == n_sc - 1:
            aoT_by_sc[sc] = aoT
        if cq == CH_PER_SC - 1 and sc < n_sc - 1:
            oproj_next.append((aoT, sc))

    # drain: last chunk's transposes interleaved with the already-available
    # out-projection groups, then the final two chunks' out-projections
    while tr_pending:
        tr_pending.pop(0)()
        if grp_q:
            grp_q.pop(0)()
    last_c = n_sc * CH_PER_SC - 1
    fsc, fcq = divmod(last_c, CH_PER_SC)
    grp_q.extend(outproj_groups(aoT_by_sc[fsc], fsc,
                                slice(fcq * T, (fcq + 1) * T)))
    while oproj_next:
        grp_q.extend(outproj_groups(*oproj_next.pop(0)))
    while grp_q:
        grp_q.pop(0)()


def host_prep(hidden_states, wq, bq, wk, bk, wv, bv, wo, bo):
    """Per-core input maps (list of 8 dicts) from the full fp32 inputs."""
    x = np.asarray(hidden_states, dtype=np.float32)
    scale = 1.0 / np.sqrt(D)
    Wq = (np.asarray(wq).T * scale).astype(NPBF16)
    Wk = np.asarray(wk).T.astype(NPBF16)
    Wv = np.asarray(wv).T.astype(NPBF16)
    Wo = np.asarray(wo).T.astype(NPBF16)
    bq_eff = np.ascontiguousarray((np.asarray(bq) * scale).reshape(MQ, 128).T).astype(np.float32)
    bo_eff = np.ascontiguousarray(
        (np.asarray(bo) + 2.0 * (np.asarray(wo) @ np.asarray(bv))).reshape(MQ, 128).T
    ).astype(np.float32)
    xbar = x.reshape(B, C, T, E).mean(axis=2)  # (B, C, E) fp32

    j_half = np.arange(C) % C  # j index vector
    cl_idx = np.arange(LCH)[None, :]
    in_maps = []
    for c in range(N_CORES):
        b, half = divmod(c, 2)
        xs = x[b, half * TOK:(half + 1) * TOK, :]
        # masksT[j_stacked, c_loc]: additive bias on inter scores, per
        # partition j (rows 0:64 = head-even j, rows 64:128 = head-odd j)
        jj = np.arange(64)[:, None]
        m64 = np.where(jj <= half * LCH + cl_idx, 1.0, 0.0)
        mT = np.concatenate([m64, m64], axis=0).astype(np.float32)  # (128, LCH)
        in_maps.append({
            "xT": xs.T.astype(NPBF16),
            "xbarT": xbar[b].T.astype(NPBF16),
            "masksT": mT,
            "wq": Wq, "wk": Wk, "wv": Wv, "wo": Wo,
            "bq": bq_eff, "bo": bo_eff,
        })
    return in_maps


_NC_CACHE = {}


def _get_nc():
    if "nc" not in _NC_CACHE:
        _NC_CACHE["nc"] = build_nc(N_SC)
    return _NC_CACHE["nc"]


def kernel(**inputs):
    in_maps = host_prep(**inputs)
    nc = _get_nc()
    res = run_bass_kernel_spmd(nc, in_maps, core_ids=list(range(N_CORES)))
    out = np.empty((B, S, E), dtype=np.float32)
    for c in range(N_CORES):
        b, half = divmod(c, 2)
        out[b, half * TOK:(half + 1) * TOK, :] = res.results[c]["outT"].T
    return out
